# revision 1
# baseline (speedup 1.0000x reference)
"""Bahdanau additive attention on 8 Trainium2 NeuronCores.

Shapes: query (8,512,128), encoder_outputs (8,512,128), src_lengths (8,)
Output: (8,512,128) float32.

Sharding (default VERSION, v5/v6 family): each core owns a 64-row T-slice
for ALL batch elements (no collectives; params + encoder replicated). The
program is specialized at build time to the actual src_lengths so tanh
work is only spent on unmasked score columns; masked columns are exact
zeros (matching the reference's quirk of zeroing, not -inf).

Per-core hot path (H=128 on partitions):
  WS^T = W_s^T.T @ q^T, per-b WH^T = W_h^T.T @ e_b^T        (PE)
  per 8-row group: DVE broadcast-adds WH^T + ws_t columns,
  one ScalarE tanh over the 8*len block,
  8 PE matmuls lhsT=(v at strip column j) accumulate score rows into
  32-row PSUM strips; a row permutation interleaves consecutive t across
  PE column-groups so the fp32 LOW_HIGH matmuls run concurrently
  (col-tiling). Softmax per 128-row pair tile (exp + row-sum fused via
  accum_out), attn^T via PE transposes, ct^T and the output head as
  plain matmul chains, final transpose + DMA.

v1 (one batch element per core, ACT-bias per t) is kept as VERSION="v1"
fallback. Measured on HW: v1 471us -> v3 353us -> v5 241us per invocation.
"""

import numpy as np

B, T, S, H = 8, 512, 512, 128
NB = 32  # psum strip width for the v-dot accumulation trick

_CACHE = {}


def _patch_tile_drain():
    """walrus in this env accepts only 1 sync-wait per Drain; Tile's final
    kernel-tail drain carries one wait per active proc. Split it into a
    chain of single-wait drains on the same engine (sequential -> same
    semantics)."""
    import concourse.tile as tile
    from concourse.vector_clock import ScopedClock

    if getattr(tile.TileContext, "_drain_split_patched", False):
        return

    def patched(self, tick_clock, wait_clock):
        drain_inst = self.nc.sync.drain()
        wait_clock.add_sem_waits(
            drain_inst.ins, ScopedClock({None: tick_clock.global_clock})
        )
        si = drain_inst.ins.sync_info
        waits = list(si.on_wait) if si else []
        if len(waits) > 1:
            si.on_wait = waits[:1]
            for w in waits[1:]:
                d2 = self.nc.sync.drain()
                d2.ins.sync_info = type(si)(on_wait=[w], on_update=[])
        self.nc.all_engine_barrier()
        popped = self.nc._tile_sem_poison_stack.pop()
        assert popped is self._sem_poison
        self.nc.clear_and_free_semaphores(list(self.sems.allocated().values()))
        self.nc.all_engine_barrier()

    tile.TileContext._drain_and_barrier = patched
    tile.TileContext._drain_split_patched = True


def _split_multi_waits(nc):
    """This env's walrus accepts only ONE sync-wait per instruction. Hoist
    extra waits onto fresh same-engine NoOps placed immediately before the
    instruction (engine streams are sequential, so semantics are identical)."""
    from concourse import mybir

    ctr = [0]
    for fn in nc.m.functions:
        for blk in fn.blocks:
            insts = blk.instructions
            if not any(
                i.sync_info is not None and len(i.sync_info.on_wait) > 1
                for i in insts
            ):
                continue
            new = []
            for inst in insts:
                si = inst.sync_info
                if si is not None and len(si.on_wait) > 1:
                    waits = list(si.on_wait)
                    for w in waits[:-1]:
                        ctr[0] += 1
                        nop = mybir.InstNoOp(
                            name=f"waitsplit-{ctr[0]}",
                            sync_info=mybir.SyncInfo(on_wait=[w], on_update=[]),
                            engine=inst.engine,
                            bass_nofuse=True,
                        )
                        nc.register_instruction(nop, overwrite=True)
                        new.append(nop)
                    si.on_wait = waits[-1:]
                new.append(inst)
            blk.instructions = new
    return ctr[0]


def _build_program():
    import concourse.bass as bass
    import concourse.tile as tile
    from concourse import mybir

    _patch_tile_drain()
    f32 = mybir.dt.float32
    AF = mybir.ActivationFunctionType

    nc = bass.Bass()
    qT_d = nc.declare_dram_parameter("qT", [H, T], f32, isOutput=False)
    e_d = nc.declare_dram_parameter("e", [S, H], f32, isOutput=False)
    eT_d = nc.declare_dram_parameter("eT", [H, S], f32, isOutput=False)
    WsT_d = nc.declare_dram_parameter("WsT", [H, H], f32, isOutput=False)
    WhT_d = nc.declare_dram_parameter("WhT", [H, H], f32, isOutput=False)
    Wo1T_d = nc.declare_dram_parameter("Wo1T", [H, H], f32, isOutput=False)
    Wo2T_d = nc.declare_dram_parameter("Wo2T", [H, H], f32, isOutput=False)
    Wob_d = nc.declare_dram_parameter("Wob", [H, 1], f32, isOutput=False)
    Vv_d = nc.declare_dram_parameter("Vv", [H, NB, NB], f32, isOutput=False)
    mask_d = nc.declare_dram_parameter("mask", [128, S], f32, isOutput=False)
    ident_d = nc.declare_dram_parameter("ident", [128, 128], f32, isOutput=False)
    out_d = nc.declare_dram_parameter("out", [T, H], f32, isOutput=True)

    with tile.TileContext(nc) as tc:
        with (
            tc.tile_pool(name="consts", bufs=1) as consts,
            tc.tile_pool(name="work", bufs=3) as work,
            tc.tile_pool(name="stats", bufs=8) as stats,
            tc.tile_pool(name="ps_big", bufs=2, space="PSUM") as ps_big,
            tc.tile_pool(name="ps_tr", bufs=2, space="PSUM") as ps_tr,
        ):
            def load(shape, src, tag):
                t = consts.tile(shape, f32, tag=tag)
                nc.sync.dma_start(out=t[:], in_=src[:])
                return t

            qT_sb = load([H, T], qT_d, "qT")
            eT_sb = load([H, S], eT_d, "eT")
            WsT_sb = load([H, H], WsT_d, "WsT")
            WhT_sb = load([H, H], WhT_d, "WhT")
            Wo1T_sb = load([H, H], Wo1T_d, "Wo1T")
            Wo2T_sb = load([H, H], Wo2T_d, "Wo2T")
            Wob_sb = load([H, 1], Wob_d, "Wob")
            Vv_sb = load([H, NB, NB], Vv_d, "Vv")
            mask_sb = load([128, S], mask_d, "mask")
            ident_sb = load([128, 128], ident_d, "ident")
            e_sb = consts.tile([128, 4, H], f32)
            for c in range(4):
                nc.sync.dma_start(out=e_sb[:, c, :], in_=e_d[c * 128:(c + 1) * 128, :])

            # WS^T (H x T) and WH^T (H x S)
            ws_ps = ps_big.tile([128, T], f32, tag="big")
            nc.tensor.matmul(ws_ps, lhsT=WsT_sb, rhs=qT_sb, start=True, stop=True)
            WS_sb = consts.tile([H, T], f32)
            nc.vector.tensor_copy(out=WS_sb, in_=ws_ps)
            wh_ps = ps_big.tile([128, S], f32, tag="big")
            nc.tensor.matmul(wh_ps, lhsT=WhT_sb, rhs=eT_sb, start=True, stop=True)
            WH_sb = consts.tile([H, S], f32)
            nc.vector.tensor_copy(out=WH_sb, in_=wh_ps)

            attn_sb = consts.tile([128, 4, S], f32)   # [t-part, t-block, s]
            attnT_sb = consts.tile([128, 4, T], f32)  # [s-part, s-chunk, t]

            for blk in range(4):
                sc_ps = ps_big.tile([128, S], f32, tag="big")
                for k in range(4):
                    for j in range(NB):
                        t = blk * 128 + k * NB + j
                        A = work.tile([128, S], f32, tag="A")
                        nc.scalar.activation(A, WH_sb, AF.Tanh, bias=WS_sb[:, t:t + 1])
                        nc.tensor.matmul(
                            sc_ps[k * NB:(k + 1) * NB, :],
                            lhsT=Vv_sb[:, j, :],
                            rhs=A,
                            start=(j == 0),
                            stop=(j == NB - 1),
                            tile_position=(0, k * NB),
                        )
                # masked softmax over S (rows = 128 t values)
                sc_sb = work.tile([128, S], f32, tag="sc")
                nc.vector.tensor_mul(out=sc_sb, in0=sc_ps, in1=mask_sb)
                neg_mx = stats.tile([128, 1], f32, tag="st")
                nc.vector.tensor_reduce(
                    out=neg_mx, in_=sc_sb, axis=mybir.AxisListType.X,
                    op=mybir.AluOpType.max, negate=True,
                )
                ex = work.tile([128, S], f32, tag="ex")
                ssum = stats.tile([128, 1], f32, tag="st")
                nc.scalar.activation(ex, sc_sb, AF.Exp, bias=neg_mx, accum_out=ssum)
                rec = stats.tile([128, 1], f32, tag="st")
                nc.vector.reciprocal(rec, ssum)
                nc.vector.tensor_scalar_mul(
                    out=attn_sb[:, blk, :], in0=ex, scalar1=rec
                )
                for c in range(4):
                    trp = ps_tr.tile([128, 128], f32, tag="tr")
                    nc.tensor.transpose(
                        trp, attn_sb[:, blk, c * 128:(c + 1) * 128], ident_sb
                    )
                    nc.vector.tensor_copy(
                        out=attnT_sb[:, c, blk * 128:(blk + 1) * 128], in_=trp
                    )

            # ct^T (H x T) = sum over s-chunks of e_chunk.T @ attn^T_chunk
            ct_ps = ps_big.tile([128, T], f32, tag="big")
            for c in range(4):
                nc.tensor.matmul(
                    ct_ps, lhsT=e_sb[:, c, :], rhs=attnT_sb[:, c, :],
                    start=(c == 0), stop=(c == 3),
                )
            ctT_sb = consts.tile([H, T], f32)
            nc.vector.tensor_copy(out=ctT_sb, in_=ct_ps)

            # out^T (H x T) = tanh(Wo1T.T @ ct^T + Wo2T.T @ q^T + b)
            o_ps = ps_big.tile([128, T], f32, tag="big")
            nc.tensor.matmul(o_ps, lhsT=Wo1T_sb, rhs=ctT_sb, start=True, stop=False)
            nc.tensor.matmul(o_ps, lhsT=Wo2T_sb, rhs=qT_sb, start=False, stop=True)
            outT_sb = consts.tile([H, T], f32)
            nc.scalar.activation(outT_sb, o_ps, AF.Tanh, bias=Wob_sb)
            for blk in range(4):
                trp = ps_tr.tile([128, 128], f32, tag="tr")
                nc.tensor.transpose(
                    trp, outT_sb[:, blk * 128:(blk + 1) * 128], ident_sb
                )
                ot = work.tile([128, 128], f32, tag="ot")
                nc.vector.tensor_copy(out=ot, in_=trp)
                nc.sync.dma_start(
                    out=out_d[blk * 128:(blk + 1) * 128, :], in_=ot
                )
    _split_multi_waits(nc)
    return nc


def _row_perm(interleave):
    """Map t_local (0..63) -> psum row r within a 64-row half. With
    interleave, consecutive t go to different 32-row strips so their
    score matmuls land in different PE column-groups and can execute
    concurrently (col-tiling)."""
    if interleave:
        return [(tl % 2) * 32 + tl // 2 for tl in range(64)]
    return list(range(64))


def _build_program_v3(lens, f32r_vdot=False, gpsimd_split=False,
                      interleave=False, act_bias_groups=0):
    """(b,t)-sharded, length-specialized program.

    Each core owns a 64-row T-slice for ALL batch elements. Per (b,t) row
    only src_lengths[b] columns of tanh are computed (masked scores are 0
    by construction via memset). tanh inputs are pre-summed on the DVE in
    groups of 8 rows so one ScalarE op covers 8*len elements.
    lens: per-batch lengths (python ints) baked into the program; same for
    every core, so the program stays SPMD.
    f32r_vdot: run the score-reduction matmuls in float32r (single-pass on
    the PE instead of fp32's LOW_HIGH two-pass; slightly reduced multiply
    precision - validate against the reference before trusting).
    gpsimd_split: alternate the per-row broadcast adds between DVE and
    GpSimd to halve the DVE stream time.
    """
    import concourse.bass as bass
    import concourse.tile as tile
    from concourse import mybir

    _patch_tile_drain()
    f32 = mybir.dt.float32
    AF = mybir.ActivationFunctionType
    TS = 64  # T-slice per core
    G = 8    # rows per ACT group

    lens = [int(x) for x in lens]
    lens_e = [min(S, l + (l & 1)) for l in lens]  # even for DVE 2x mode

    nc = bass.Bass()
    qT_d = nc.declare_dram_parameter("qT", [H, B * TS], f32, isOutput=False)
    e_d = nc.declare_dram_parameter("e", [B, S, H], f32, isOutput=False)
    eT_d = nc.declare_dram_parameter("eT", [B, H, S], f32, isOutput=False)
    WsT_d = nc.declare_dram_parameter("WsT", [H, H], f32, isOutput=False)
    WhT_d = nc.declare_dram_parameter("WhT", [H, H], f32, isOutput=False)
    Wo1T_d = nc.declare_dram_parameter("Wo1T", [H, H], f32, isOutput=False)
    Wo2T_d = nc.declare_dram_parameter("Wo2T", [H, H], f32, isOutput=False)
    Wob_d = nc.declare_dram_parameter("Wob", [H, 1], f32, isOutput=False)
    Vv_d = nc.declare_dram_parameter("Vv", [H, NB, NB], f32, isOutput=False)
    ident_d = nc.declare_dram_parameter("ident", [128, 128], f32, isOutput=False)
    out_d = nc.declare_dram_parameter("out", [B * TS, H], f32, isOutput=True)

    with tile.TileContext(nc) as tc:
        with (
            tc.tile_pool(name="consts", bufs=1) as consts,
            tc.tile_pool(name="work", bufs=2) as work,
            tc.tile_pool(name="work1", bufs=1) as work1,
            tc.tile_pool(name="stats", bufs=8) as stats,
            tc.tile_pool(name="ps_big", bufs=2, space="PSUM") as ps_big,
            tc.tile_pool(name="ps_tr", bufs=2, space="PSUM") as ps_tr,
        ):
            def load(shape, src, tag):
                t = consts.tile(shape, f32, tag=tag)
                nc.sync.dma_start(out=t[:], in_=src[:])
                return t

            qT_sb = load([H, B * TS], qT_d, "qT")
            WsT_sb = load([H, H], WsT_d, "WsT")
            WhT_sb = load([H, H], WhT_d, "WhT")
            Wo1T_sb = load([H, H], Wo1T_d, "Wo1T")
            Wo2T_sb = load([H, H], Wo2T_d, "Wo2T")
            Wob_sb = load([H, 1], Wob_d, "Wob")
            Vv_sb = load([H, NB, NB], Vv_d, "Vv")
            ident_sb = load([128, 128], ident_d, "ident")
            e_sb = consts.tile([128, B, 4, H], f32)   # encoder, s on partitions
            eT_sb = consts.tile([H, B, S], f32)       # encoder^T, h on partitions
            for b in range(B):
                nc.sync.dma_start(out=eT_sb[:, b, :], in_=eT_d[b])
            for b in range(B):
                for c in range(4):
                    nc.gpsimd.dma_start(
                        out=e_sb[:, b, c, :], in_=e_d[b, c * 128:(c + 1) * 128, :]
                    )

            # WS^T for all (b, t_local) columns at once
            ws_ps = ps_big.tile([128, B * TS], f32, tag="big")
            nc.tensor.matmul(ws_ps, lhsT=WsT_sb, rhs=qT_sb, start=True, stop=True)
            WS_sb = consts.tile([H, B * TS], f32)
            nc.vector.tensor_copy(out=WS_sb, in_=ws_ps)

            # WH^T per batch element (only len columns matter)
            WH_sb = consts.tile([H, B, S], f32)
            for b in range(B):
                wh_ps = ps_big.tile([128, S], f32, tag="big")
                nc.tensor.matmul(
                    wh_ps[:, :lens_e[b]], lhsT=WhT_sb,
                    rhs=eT_sb[:, b, :lens_e[b]], start=True, stop=True,
                )
                nc.vector.tensor_copy(
                    out=WH_sb[:, b, :lens_e[b]], in_=wh_ps[:, :lens_e[b]]
                )

            attn_sb = consts.tile([128, 4, S], f32)   # [pair-rows, pair, s]
            attnT_sb = consts.tile([128, 4, B * TS], f32)  # [s, s-chunk, col]
            perm = _row_perm(interleave)

            fourway = interleave == 4
            for pair in range(4):
                sc_ps = ps_big.tile([128, S], f32, tag="big")
                if fourway:
                    # alternate the pair's two halves per group: consecutive
                    # score matmuls hit 4 distinct PE column strips.
                    for g in range(TS // G):
                        A8s = {}
                        for half in range(2):
                            b = pair * 2 + half
                            le = lens_e[b]
                            SUMg = work1.tile([128, G, S], f32,
                                              tag=f"SUM{half}")
                            for j in range(G):
                                tl = g * G + j
                                col = b * TS + perm[tl]
                                eng = (nc.gpsimd if (gpsimd_split and j % 2)
                                       else nc.vector)
                                eng.tensor_scalar_add(
                                    out=SUMg[:, j, :le],
                                    in0=WH_sb[:, b, :le],
                                    scalar1=WS_sb[:, col:col + 1],
                                )
                            A8 = work.tile([128, G, S], f32, tag=f"A8{half}")
                            nc.scalar.activation(
                                A8[:, :, :le], SUMg[:, :, :le], AF.Tanh
                            )
                            A8s[half] = A8
                        for j in range(G):
                            tl = g * G + j
                            for half in range(2):
                                b = pair * 2 + half
                                ln = lens[b]
                                row = half * TS + perm[tl]
                                k = row // NB
                                jj = row % NB
                                nc.tensor.matmul(
                                    sc_ps[k * NB:(k + 1) * NB, :ln],
                                    lhsT=Vv_sb[:, jj, :],
                                    rhs=A8s[half][:, j, :ln],
                                    start=(jj == 0),
                                    stop=(jj == NB - 1),
                                    tile_position=(0, k * NB),
                                    skip_group_check=True,
                                )
                    ln = None
                else:
                    for half in range(2):
                        b = pair * 2 + half
                        ln, le = lens[b], lens_e[b]
                        for g in range(TS // G):
                            # last group per b takes the ScalarE-bias path
                            # (no DVE adds) to balance DVE vs ACT load
                            bias_path = act_bias_groups and g >= (
                                TS // G - act_bias_groups)
                            if bias_path:
                                for j in range(G):
                                    tl = g * G + j
                                    col = b * TS + perm[tl]
                                    Ab = work.tile([128, S], f32, tag="Ab")
                                    nc.scalar.activation(
                                        Ab[:, :ln], WH_sb[:, b, :ln], AF.Tanh,
                                        bias=WS_sb[:, col:col + 1],
                                    )
                                    row = half * TS + perm[tl]
                                    k = row // NB
                                    jj = row % NB
                                    nc.tensor.matmul(
                                        sc_ps[k * NB:(k + 1) * NB, :ln],
                                        lhsT=Vv_sb[:, jj, :],
                                        rhs=Ab[:, :ln],
                                        start=(jj == 0),
                                        stop=(jj == NB - 1),
                                        tile_position=(0, k * NB),
                                        skip_group_check=bool(interleave),
                                    )
                                continue
                            SUMg = work.tile([128, G, S], f32, tag="SUM")
                            for j in range(G):
                                tl = g * G + j
                                col = b * TS + perm[tl]
                                eng = (nc.gpsimd if (gpsimd_split and j % 2)
                                       else nc.vector)
                                eng.tensor_scalar_add(
                                    out=SUMg[:, j, :le],
                                    in0=WH_sb[:, b, :le],
                                    scalar1=WS_sb[:, col:col + 1],
                                )
                            A8 = work.tile([128, G, S], f32, tag="A8")
                            nc.scalar.activation(
                                A8[:, :, :le], SUMg[:, :, :le], AF.Tanh
                            )
                            for j in range(G):
                                tl = g * G + j          # t_local 0..63
                                row = half * TS + perm[tl]
                                k = row // NB
                                jj = row % NB
                                nc.tensor.matmul(
                                    sc_ps[k * NB:(k + 1) * NB, :ln],
                                    lhsT=Vv_sb[:, jj, :],
                                    rhs=A8[:, j, :ln],
                                    start=(jj == 0),
                                    stop=(jj == NB - 1),
                                    tile_position=(0, k * NB),
                                    skip_group_check=bool(interleave),
                                )
                # masked softmax rows of this pair
                sc_sb = work.tile([128, S], f32, tag="sc")
                for half in range(2):
                    b = pair * 2 + half
                    ln = lens[b]
                    rows = slice(half * TS, half * TS + TS)
                    nc.vector.tensor_copy(
                        out=sc_sb[rows, :ln], in_=sc_ps[rows, :ln]
                    )
                    if ln < S:
                        nc.vector.memset(sc_sb[rows, ln:], 0.0)
                neg_mx = stats.tile([128, 1], f32, tag="st")
                nc.vector.tensor_reduce(
                    out=neg_mx, in_=sc_sb, axis=mybir.AxisListType.X,
                    op=mybir.AluOpType.max, negate=True,
                )
                ex = work.tile([128, S], f32, tag="ex")
                ssum = stats.tile([128, 1], f32, tag="st")
                nc.scalar.activation(ex, sc_sb, AF.Exp, bias=neg_mx, accum_out=ssum)
                rec = stats.tile([128, 1], f32, tag="st")
                nc.vector.reciprocal(rec, ssum)
                nc.vector.tensor_scalar_mul(
                    out=attn_sb[:, pair, :], in0=ex, scalar1=rec
                )
                for c in range(4):
                    trp = ps_tr.tile([128, 128], f32, tag="tr")
                    nc.tensor.transpose(
                        trp, attn_sb[:, pair, c * 128:(c + 1) * 128], ident_sb
                    )
                    nc.vector.tensor_copy(
                        out=attnT_sb[:, c, pair * 128:(pair + 1) * 128], in_=trp
                    )

            # ct^T columns (global col = b*TS + t_local)
            ct_ps = ps_big.tile([128, B * TS], f32, tag="big")
            for b in range(B):
                cols = slice(b * TS, (b + 1) * TS)
                for c in range(4):
                    nc.tensor.matmul(
                        ct_ps[:, cols], lhsT=e_sb[:, b, c, :],
                        rhs=attnT_sb[:, c, cols],
                        start=(c == 0), stop=(c == 3),
                    )
            ctT_sb = consts.tile([H, B * TS], f32)
            nc.vector.tensor_copy(out=ctT_sb, in_=ct_ps)

            o_ps = ps_big.tile([128, B * TS], f32, tag="big")
            nc.tensor.matmul(o_ps, lhsT=Wo1T_sb, rhs=ctT_sb, start=True, stop=False)
            nc.tensor.matmul(o_ps, lhsT=Wo2T_sb, rhs=qT_sb, start=False, stop=True)
            outT_sb = consts.tile([H, B * TS], f32)
            nc.scalar.activation(outT_sb, o_ps, AF.Tanh, bias=Wob_sb)
            for blk in range(4):
                trp = ps_tr.tile([128, 128], f32, tag="tr")
                nc.tensor.transpose(
                    trp, outT_sb[:, blk * 128:(blk + 1) * 128], ident_sb
                )
                ot = work.tile([128, 128], f32, tag="ot")
                nc.vector.tensor_copy(out=ot, in_=trp)
                nc.sync.dma_start(
                    out=out_d[blk * 128:(blk + 1) * 128, :], in_=ot
                )
    _split_multi_waits(nc)
    return nc


def _host_prep_v3(query, encoder_outputs, src_lengths, W_h, W_s, v,
                  W_out_w, W_out_b, interleave=False):
    f = np.float32
    TS = 64
    perm = np.array(_row_perm(interleave))
    query = np.asarray(query, f)
    enc = np.asarray(encoder_outputs, f)
    W_h = np.asarray(W_h, f)
    W_s = np.asarray(W_s, f)
    v = np.asarray(v, f)
    W_out_w = np.asarray(W_out_w, f)
    W_out_b = np.asarray(W_out_b, f)

    WsT = np.ascontiguousarray(W_s.T)
    WhT = np.ascontiguousarray(W_h.T)
    Wo1T = np.ascontiguousarray(W_out_w[:, :H].T)
    Wo2T = np.ascontiguousarray(W_out_w[:, H:].T)
    Wob = np.ascontiguousarray(W_out_b.reshape(H, 1))
    Vv = np.zeros((H, NB, NB), f)
    for j in range(NB):
        Vv[:, j, j] = v
    ident = np.eye(128, dtype=f)
    e_all = np.ascontiguousarray(enc)                      # (B,S,H)
    eT_all = np.ascontiguousarray(enc.transpose(0, 2, 1))  # (B,H,S)

    in_maps = []
    for ci in range(B):
        qs = query[:, ci * TS:(ci + 1) * TS, :]            # (B,TS,H)
        qs_p = np.empty_like(qs)
        qs_p[:, perm, :] = qs                              # col r holds t=inv[r]
        qT = np.ascontiguousarray(
            qs_p.transpose(2, 0, 1).reshape(H, B * TS))    # (H, B*TS)
        in_maps.append({
            "qT": qT, "e": e_all, "eT": eT_all,
            "WsT": WsT, "WhT": WhT, "Wo1T": Wo1T, "Wo2T": Wo2T,
            "Wob": Wob, "Vv": Vv, "ident": ident,
        })
    return in_maps


import os as _os
VERSION = _os.environ.get("BAHDANAU_VERSION", "v5")


def _get_program(lens=None):
    if VERSION in ("v3", "v3g", "v4", "v5", "v6", "v7"):
        key = (VERSION, tuple(int(x) for x in lens))
        if key not in _CACHE:
            # gpsimd_split measured 5x SLOWER on HW (GpSimd tensor_scalar
            # ~20x DVE cost) - only kept for the v3g experiment.
            _CACHE[key] = _build_program_v3(
                lens,
                f32r_vdot=(VERSION == "v4"),
                gpsimd_split=(VERSION == "v3g"),
                interleave=(4 if VERSION == "v6"
                            else VERSION in ("v5", "v7")),
                act_bias_groups=(1 if VERSION == "v7" else 0),
            )
        return _CACHE[key]
    if "nc" not in _CACHE:
        _CACHE["nc"] = _build_program()
    return _CACHE["nc"]


def _host_prep(query, encoder_outputs, src_lengths, W_h, W_s, v,
               W_out_w, W_out_b):
    f = np.float32
    query = np.asarray(query, f)
    enc = np.asarray(encoder_outputs, f)
    lens = np.asarray(src_lengths).astype(np.int64)
    W_h = np.asarray(W_h, f)
    W_s = np.asarray(W_s, f)
    v = np.asarray(v, f)
    W_out_w = np.asarray(W_out_w, f)
    W_out_b = np.asarray(W_out_b, f)

    WsT = np.ascontiguousarray(W_s.T)
    WhT = np.ascontiguousarray(W_h.T)
    Wo1T = np.ascontiguousarray(W_out_w[:, :H].T)
    Wo2T = np.ascontiguousarray(W_out_w[:, H:].T)
    Wob = np.ascontiguousarray(W_out_b.reshape(H, 1))
    Vv = np.zeros((H, NB, NB), f)
    for j in range(NB):
        Vv[:, j, j] = v
    ident = np.eye(128, dtype=f)

    in_maps = []
    for b in range(B):
        mask_row = (np.arange(S) < int(lens[b])).astype(f)
        in_maps.append({
            "qT": np.ascontiguousarray(query[b].T),
            "e": np.ascontiguousarray(enc[b]),
            "eT": np.ascontiguousarray(enc[b].T),
            "WsT": WsT, "WhT": WhT, "Wo1T": Wo1T, "Wo2T": Wo2T,
            "Wob": Wob, "Vv": Vv,
            "mask": np.ascontiguousarray(np.broadcast_to(mask_row, (128, S))),
            "ident": ident,
        })
    return in_maps


def _prep_for_run(inputs):
    """Returns (nc, in_maps) for the current VERSION. Used by test harness."""
    if VERSION in ("v3", "v3g", "v4", "v5", "v6", "v7"):
        lens = np.asarray(inputs["src_lengths"]).astype(np.int64)
        return _get_program(lens), _host_prep_v3(interleave=(VERSION in ("v5", "v6", "v7")), **inputs)
    return _get_program(), _host_prep(**inputs)


def kernel(query, encoder_outputs, src_lengths, W_h, W_s, v, W_out_w,
           W_out_b):
    from concourse.bass_utils import run_bass_kernel_spmd

    lens = np.asarray(src_lengths).astype(np.int64)
    if VERSION in ("v3", "v3g", "v4", "v5", "v6", "v7"):
        TS = 64
        perm = np.array(_row_perm(VERSION in ("v5", "v6", "v7")))
        nc = _get_program(lens)
        in_maps = _host_prep_v3(query, encoder_outputs, src_lengths, W_h,
                                W_s, v, W_out_w, W_out_b,
                                interleave=(VERSION in ("v5", "v6", "v7")))
        res = run_bass_kernel_spmd(nc, in_maps, list(range(B)))
        out = np.empty((B, T, H), np.float32)
        for ci in range(B):
            o = np.asarray(res.results[ci]["out"]).reshape(B, TS, H)
            out[:, ci * TS:(ci + 1) * TS, :] = o[:, perm, :]
        return out
    nc = _get_program()
    in_maps = _host_prep(query, encoder_outputs, src_lengths, W_h, W_s, v,
                         W_out_w, W_out_b)
    res = run_bass_kernel_spmd(nc, in_maps, list(range(B)))
    out = np.stack([np.asarray(res.results[b]["out"]) for b in range(B)])
    return out.astype(np.float32)


if __name__ == "__main__":
    rng = np.random.default_rng(0)
    ins = {
        "query": rng.standard_normal((B, T, H)).astype(np.float32),
        "encoder_outputs": rng.standard_normal((B, S, H)).astype(np.float32),
        "src_lengths": np.concatenate([[S], rng.integers(1, S + 1, B - 1)]),
        "W_h": rng.standard_normal((H, H)).astype(np.float32) * (H ** -0.5),
        "W_s": rng.standard_normal((H, H)).astype(np.float32) * (H ** -0.5),
        "v": rng.standard_normal(H).astype(np.float32) * (H ** -0.5),
        "W_out_w": rng.standard_normal((H, 2 * H)).astype(np.float32) * ((2 * H) ** -0.5),
        "W_out_b": rng.standard_normal(H).astype(np.float32) * 0.01,
    }
    out = kernel(**ins)
    print("kernel output", out.shape, out.dtype)

